# revision 10
# baseline (speedup 1.0000x reference)
"""Trainium2 Bass kernel for BlockSoftmaxLinearHybrid.

The warm-call wall time is dominated by the axon tunnel (~78MB/s up,
~55MB/s down, full duplex), so the host<->device path is organized to
move the minimum number of bytes and overlap transfers:

  * inputs ship in natural (L, D) layout as bf16 -- no host transposes;
    the kernel transposes Q/K on device with the DMA XBAR and pads V
    with a ones column on device.
  * output ships as fp16 (error budget allows it).
  * the jitted shard_map dispatch and the zero output operands are
    cached across calls (zeros are not donated; the kernel writes every
    output element, so the result buffer needs no pre-zeroing).
  * work is split into NCALL chunks of (B*H)/NCALL pairs, pipelined so
    chunk j's download overlaps chunk j+1's upload.

Device kernel per (b,h) pair:
  phase A: u_q^T = W^T Q^T (f-major), EXPQ=[exp(u);exp(-u)] unnormalized
           (normalization recovered via ones-column in the state matmul);
           u_k in natural layout, exp'd and row-normalized -> phi_k.
  phase B: per 64-row block scan: block-local softmax attention
           (scores^T -> exp -> @[v|1]) + linear attention vs the running
           [S|Z] state accumulated in PSUM, blended with w=sigmoid(alpha).
"""

import sys

import numpy as np

if "/opt/trn_rl_repo" not in sys.path:
    sys.path.insert(0, "/opt/trn_rl_repo")

import ml_dtypes

import concourse.bass as bass
import concourse.bacc as bacc
import concourse.mybir as mybir
from concourse.tile import TileContext

B, H, L, D = 2, 16, 4096, 128
F = 64          # feature dim; phi dim is 2F = 128
SBLK = 64       # block size
NBLK = L // SBLK            # 64 blocks
NCH = L // 128              # 32 chunks (2 blocks each)
EPS = 1e-6
SCALING = D ** -0.5
NCORES = 8
PAIRS = B * H               # 32 (b,h) pairs
NCALL = 2                   # pipeline chunks per kernel() call
RPC = PAIRS // NCALL        # pairs per chunk (rows of the global array)
PPC = RPC // NCORES         # pairs per core per chunk
NGRP = NCH                  # phase-B group count

BF16 = mybir.dt.bfloat16
F16 = mybir.dt.float16
F32 = mybir.dt.float32
INT8 = mybir.dt.int8
AX = mybir.AxisListType
ALU = mybir.AluOpType
ACTF = mybir.ActivationFunctionType


def _bcast_last(ap, n):
    """Append a stride-0 dim of size n to an AP (free-dim broadcast)."""
    return bass.AP(tensor=ap.tensor, offset=ap.offset, ap=list(ap.ap) + [[0, n]])


def build_nc(w: float) -> bass.Bass:
    nc = bacc.Bacc()

    qk_d = nc.dram_tensor("qk", [2, PPC, L, D], BF16, kind="ExternalInput")
    # v rows quantized to int8 with a per-row bf16 scale packed in the
    # trailing 2 bytes (cuts v upload bytes in half; error impact is
    # negligible because attention averages v).
    vq_d = nc.dram_tensor("vq", [PPC, L, 130], INT8, kind="ExternalInput")
    wh_d = nc.dram_tensor("wh", [PPC, 128, F], BF16, kind="ExternalInput")
    qn_d, kn_d = qk_d[0], qk_d[1]
    out_d = nc.dram_tensor("out", [PPC, NCH, 128, D], F16, kind="ExternalOutput")

    with TileContext(nc) as tc:
        with (
            tc.tile_pool(name="sb", bufs=2) as sb,
            tc.tile_pool(name="small", bufs=2) as small,
            tc.tile_pool(name="grp", bufs=3) as grp,
            tc.tile_pool(name="pA", bufs=1, space="PSUM") as pA,
            tc.tile_pool(name="pSO", bufs=1, space="PSUM") as pSO,
            tc.tile_pool(name="pLQ", bufs=2, space="PSUM") as pLQ,
            tc.tile_pool(name="pST", bufs=2, space="PSUM") as pST,
        ):
            for i in range(PPC):
                # ---- load pair inputs (Q/K transposed by the DMA XBAR) ----
                qt = sb.tile([128, L], BF16, tag="qt")
                nc.sync.dma_start_transpose(out=qt, in_=qn_d[i])
                kt = sb.tile([128, L], BF16, tag="kt")
                nc.sync.dma_start_transpose(out=kt, in_=kn_d[i])
                vai = sb.tile([128, NCH, 130], INT8, tag="vai")
                nc.sync.dma_start(
                    out=vai,
                    in_=vq_d[i].rearrange("(c p) k -> p c k", p=128))
                va = sb.tile([128, NCH, 130], BF16, tag="va")
                nc.vector.memset(va[:, :, 128:129], 1.0)
                vsc = vai[:, :, 128:130].bitcast(BF16)
                nc.vector.scalar_tensor_tensor(
                    va[:, :, 0:128], vai[:, :, 0:128], 1.0,
                    _bcast_last(vsc[:, :, 0], 128),
                    op0=ALU.mult, op1=ALU.mult)
                whs = small.tile([128, F], BF16, tag="wh")
                nc.sync.dma_start(out=whs, in_=wh_d[i])

                expq = sb.tile([128, L], BF16, tag="expq")
                expk = sb.tile([128, NCH, 128], BF16, tag="expk")
                phik = sb.tile([128, NCH, 128], BF16, tag="phik")
                outst = sb.tile([128, NCH, D], F16, tag="outst")

                # ---- phase A: q features (f-major, unnormalized) ----
                for j in range(8):
                    pu = pA.tile([128, 512], F32, tag="mm")
                    nc.tensor.matmul(
                        pu[0:64, :], lhsT=whs, rhs=qt[:, j * 512:(j + 1) * 512],
                        start=True, stop=True,
                    )
                    nc.scalar.activation(
                        expq[0:64, j * 512:(j + 1) * 512], pu[0:64, :], ACTF.Exp)
                    nc.scalar.activation(
                        expq[64:128, j * 512:(j + 1) * 512], pu[0:64, :], ACTF.Exp,
                        scale=-1.0)

                # ---- phase A: k features (natural layout) ----
                for jj in range(4):
                    pk = pA.tile([128, 512], F32, tag="mm")
                    for c8 in range(8):
                        c = jj * 8 + c8
                        nc.tensor.matmul(
                            pk[:, c8 * 64:(c8 + 1) * 64],
                            lhsT=kt[:, c * 128:(c + 1) * 128], rhs=whs,
                            start=True, stop=True,
                        )
                    pk3 = pk.rearrange("p (c f) -> p c f", f=64)
                    nc.scalar.activation(
                        expk[:, jj * 8:(jj + 1) * 8, 0:64], pk3, ACTF.Exp)
                    nc.scalar.activation(
                        expk[:, jj * 8:(jj + 1) * 8, 64:128], pk3, ACTF.Exp,
                        scale=-1.0)

                # normalize phi_k rows (per 64-feature half)
                sums = small.tile([128, NCH, 2], F32, tag="sums")
                nc.vector.tensor_reduce(
                    sums, expk.rearrange("p c (t f) -> p c t f", f=64),
                    axis=AX.X, op=ALU.add)
                recs = small.tile([128, NCH, 2], F32, tag="recs")
                nc.vector.reciprocal(recs, sums)
                for c in range(NCH):
                    for t in range(2):
                        nc.vector.tensor_scalar_mul(
                            phik[:, c, t * 64:(t + 1) * 64],
                            expk[:, c, t * 64:(t + 1) * 64],
                            recs[:, c, t:t + 1])

                # ---- phase B: block scan ----
                state = small.tile([128, 130], BF16, tag="state")
                nc.vector.memset(state[:, 0:129], 0.0)
                nc.vector.memset(state[:, 129:130], 1.0)
                sps_t = pST.tile([128, 512], F32, tag="st")
                sps = sps_t[:, 0:129]

                for g in range(NGRP):
                    c0, c1 = g * 128, (g + 1) * 128
                    # block-pair scores^T and exp
                    psc = pA.tile([128, 512], F32, tag="mm")
                    nc.tensor.matmul(
                        psc[:, 0:128], lhsT=kt[:, c0:c1], rhs=qt[:, c0:c1],
                        start=True, stop=True)
                    sst = grp.tile([128, 128], BF16, tag="sst")
                    nc.scalar.activation(sst, psc[:, 0:128], ACTF.Exp, scale=SCALING)

                    pso_t = pSO.tile([128, 512], F32, tag="so")
                    pso = pso_t[:, 0:129]
                    plq1_t = pLQ.tile([128, 512], F32, tag="lq1")
                    plq1 = plq1_t[:, 0:130]
                    plq2_t = pLQ.tile([128, 512], F32, tag="lq2")
                    plq2 = plq2_t[:, 0:130]

                    for h in range(2):  # even / odd block in the chunk
                        r0, r1 = h * 64, h * 64 + 64
                        # in-block softmax numerator @ [v|1]
                        nc.tensor.matmul(
                            pso[r0:r1, :], lhsT=sst[r0:r1, r0:r1],
                            rhs=va[r0:r1, g, 0:129],
                            start=True, stop=True, tile_position=(r0, r0))
                        # linear attention vs state (E and R halves)
                        nc.tensor.matmul(
                            plq1[r0:r1, 0:130],
                            lhsT=expq[0:64, c0 + h * 64: c0 + h * 64 + 64],
                            rhs=state[0:64, :],
                            start=True, stop=True, tile_position=(0, r0))
                        nc.tensor.matmul(
                            plq2[r0:r1, 0:130],
                            lhsT=expq[64:128, c0 + h * 64: c0 + h * 64 + 64],
                            rhs=state[64:128, :],
                            start=True, stop=True, tile_position=(64, r0))
                        # state update S += phi_k^T [v|1]
                        nc.tensor.matmul(
                            sps, lhsT=phik[r0:r1, g, :], rhs=va[r0:r1, g, 0:129],
                            start=(g == 0 and h == 0),
                            stop=(g == NGRP - 1 and h == 1),
                            skip_group_check=True,
                            tile_position=(r0, 0))
                        # refresh SBUF state copy for the next block
                        if not (g == NGRP - 1 and h == 1):
                            nc.scalar.copy(state[:, 0:129], sps)

                    # ---- assembly for the two blocks of this chunk ----
                    rs = grp.tile([128, 6], F32, tag="rs")
                    den = grp.tile([128, 2], F32, tag="den")
                    sc = grp.tile([128, 5], F32, tag="sc")
                    soev = grp.tile([128, 129], F32, tag="soev")
                    nc.scalar.copy(soev, pso)
                    lqev = grp.tile([128, 260], F32, tag="lqev")
                    nc.scalar.copy(lqev[:, 0:130], plq1)
                    nc.scalar.copy(lqev[:, 130:260], plq2)
                    nc.scalar.copy(sc[:, 0:1], soev[:, 128:129])
                    nc.scalar.copy(sc[:, 1:3], lqev[:, 128:130])
                    nc.scalar.copy(sc[:, 3:5], lqev[:, 258:260])
                    nc.vector.reciprocal(rs[:, 0:1], sc[:, 0:1])
                    nc.vector.reciprocal(rs[:, 1:2], sc[:, 2:3])
                    nc.vector.reciprocal(rs[:, 2:3], sc[:, 4:5])
                    nc.vector.tensor_scalar_mul(den[:, 0:1], sc[:, 1:2],
                                                rs[:, 1:2])
                    nc.vector.scalar_tensor_tensor(
                        den[:, 1:2], sc[:, 3:4], rs[:, 2:3], den[:, 0:1],
                        op0=ALU.mult, op1=ALU.add)
                    nc.vector.tensor_scalar_max(den[:, 0:1], den[:, 1:2], EPS)
                    nc.vector.reciprocal(rs[:, 3:4], den[:, 0:1])
                    nc.vector.tensor_scalar_mul(rs[:, 4:5], rs[:, 3:4], 1.0 - w)
                    nc.vector.tensor_scalar_mul(rs[:, 5:6], rs[:, 0:1], w)

                    t2 = grp.tile([128, 128], F32, tag="t2")
                    nc.vector.tensor_scalar_mul(t2, lqev[:, 0:128], rs[:, 1:2])
                    lin = grp.tile([128, 128], F32, tag="lin")
                    nc.vector.scalar_tensor_tensor(
                        lin, lqev[:, 130:258], rs[:, 2:3], t2,
                        op0=ALU.mult, op1=ALU.add)
                    sofl = grp.tile([128, 128], F32, tag="sofl")
                    nc.vector.tensor_scalar_mul(sofl, soev[:, 0:128], rs[:, 5:6])
                    nc.vector.scalar_tensor_tensor(
                        outst[:, g, :], lin, rs[:, 4:5], sofl,
                        op0=ALU.mult, op1=ALU.add)

                nc.sync.dma_start(out=out_d[i].rearrange("c p e -> p c e"),
                                  in_=outst)

    nc.compile()
    return nc


_STATE = {}


def _build_state(w: float):
    import jax
    from jax.sharding import Mesh, PartitionSpec, NamedSharding
    from jax.experimental.shard_map import shard_map
    from concourse.bass2jax import (
        _bass_exec_p, install_neuronx_cc_hook, partition_id_tensor)

    nc = build_nc(w)
    install_neuronx_cc_hook()

    partition_name = (
        nc.partition_id_tensor.name if nc.partition_id_tensor else None)
    in_names, out_names, out_avals = [], [], []
    for alloc in nc.m.functions[0].allocations:
        if not isinstance(alloc, mybir.MemoryLocationSet):
            continue
        name = alloc.memorylocations[0].name
        if alloc.kind == "ExternalInput":
            if name != partition_name:
                in_names.append(name)
        elif alloc.kind == "ExternalOutput":
            out_names.append(name)
            out_avals.append(jax.core.ShapedArray(
                tuple(alloc.tensor_shape), mybir.dt.np(alloc.dtype)))
    assert in_names == ["qk", "vq", "wh"], in_names
    assert out_names == ["out"], out_names
    n_params = len(in_names)
    n_outs = len(out_names)
    all_in_names = list(in_names) + list(out_names)
    if partition_name is not None:
        all_in_names.append(partition_name)

    def _body(*args):
        operands = list(args)
        if partition_name is not None:
            operands.append(partition_id_tensor())
        outs = _bass_exec_p.bind(
            *operands,
            out_avals=tuple(out_avals),
            in_names=tuple(all_in_names),
            out_names=tuple(out_names),
            lowering_input_output_aliases=(),
            sim_require_finite=True,
            sim_require_nnan=True,
            nc=nc,
        )
        return tuple(outs)

    devices = jax.devices()[:NCORES]
    mesh = Mesh(np.asarray(devices), ("core",))
    spec = NamedSharding(mesh, PartitionSpec("core"))
    fn = jax.jit(
        shard_map(_body, mesh=mesh,
                  in_specs=(PartitionSpec("core"),) * (n_params + n_outs),
                  out_specs=(PartitionSpec("core"),) * n_outs,
                  check_rep=False),
        keep_unused=True,
    )
    # Output operands exist only to satisfy the HLO signature (the NEFF's
    # result buffers are bound separately and every element is written by
    # the kernel), so one cached device-resident zero array serves every
    # call with no per-call upload.
    zeros = [
        jax.device_put(
            np.zeros((NCORES * a.shape[0], *a.shape[1:]), a.dtype), spec)
        for a in out_avals
    ]
    return {"fn": fn, "spec": spec, "zeros": zeros}


def _get_state(w: float):
    key = round(w, 10)
    if key not in _STATE:
        _STATE[key] = _build_state(w)
    return _STATE[key]


def kernel(query_states, key_states, value_states, hedgehog_weights, alpha):
    q = np.asarray(query_states, dtype=np.float32)
    k = np.asarray(key_states, dtype=np.float32)
    v = np.asarray(value_states, dtype=np.float32)
    wts = np.asarray(hedgehog_weights, dtype=np.float32)
    a = float(np.asarray(alpha))
    w = float(1.0 / (1.0 + np.exp(-a)))

    try:
        return _run_device(q, k, v, wts, w)
    except Exception:
        import traceback
        traceback.print_exc(file=sys.stderr)
        return _host_reference(q, k, v, wts, w)


def _pack_qk(q, k, j):
    """One host pass: cast fp32->bf16 while interleaving q/k per core."""
    bf = ml_dtypes.bfloat16
    pack = np.empty((NCORES, 2, PPC, L, D), dtype=bf)
    sl = slice(j * RPC, (j + 1) * RPC)
    pack[:, 0] = q[sl].reshape(NCORES, PPC, L, D)
    pack[:, 1] = k[sl].reshape(NCORES, PPC, L, D)
    return pack.reshape(NCORES * 2, PPC, L, D)


def _pack_v(v, j):
    """Quantize v rows to int8 + per-row bf16 scale (trailing 2 bytes)."""
    bf = ml_dtypes.bfloat16
    sl = slice(j * RPC, (j + 1) * RPC)
    vv = v[sl]                                     # fp32 [RPC, L, D]
    s = (np.abs(vv).max(-1) / 127.0 + 1e-30).astype(bf)
    inv = 1.0 / s.astype(np.float32)
    t = vv * inv[..., None]
    np.rint(t, out=t)
    np.clip(t, -127.0, 127.0, out=t)
    pack = np.empty((RPC, L, 130), dtype=np.int8)
    pack[:, :, 0:128] = t
    pack[:, :, 128:130] = s.view(np.int8).reshape(RPC, L, 2)
    return pack


def _run_device(q, k, v, wts, w):
    import jax
    from concurrent.futures import ThreadPoolExecutor

    st = _get_state(w)
    qf = q.reshape(PAIRS, L, D)
    kf = k.reshape(PAIRS, L, D)
    vf = v.reshape(PAIRS, L, D)
    # head index of global row r of chunk j is (j*RPC + r) % H: identical
    # for all chunks because RPC is a multiple of H (or H divides j*RPC).
    wh = np.asarray(
        wts[(np.arange(RPC)) % H], dtype=ml_dtypes.bfloat16)
    wh_dev = jax.device_put(wh.reshape(NCORES * PPC, 128, F), st["spec"])

    # pipeline: put_j / exec_j issued in order so chunk j's execution is
    # not queued behind chunk j+1's upload; fetches run on a worker thread
    # and overlap later uploads (the tunnel is full duplex).
    fetches = []
    with ThreadPoolExecutor(max_workers=2) as ex:
        for j in range(NCALL):
            qk_dev = jax.device_put(_pack_qk(qf, kf, j), st["spec"])
            v_dev = jax.device_put(_pack_v(vf, j), st["spec"])
            o = st["fn"](qk_dev, v_dev, wh_dev, *st["zeros"])[0]
            fetches.append(ex.submit(np.asarray, o))
        parts = [f.result() for f in fetches]

    res = np.empty((PAIRS, NCH, 128, D), dtype=np.float32)
    for j, p in enumerate(parts):
        res[j * RPC:(j + 1) * RPC] = p
    return res.reshape(B, H, L, D)


def _host_reference(q, k, v, wts, w):
    # Last-resort fallback so a transient device failure still returns
    # a correct result; mirrors the block-scan math in fp32 numpy.
    out = np.empty((B, H, L, D), dtype=np.float32)
    for b in range(B):
        for h in range(H):
            u = q[b, h].reshape(NBLK, SBLK, D) @ wts[h]
            pq = np.concatenate([_sm(u), _sm(-u)], -1)
            uk = k[b, h].reshape(NBLK, SBLK, D) @ wts[h]
            pk = np.concatenate([_sm(uk), _sm(-uk)], -1)
            vb = v[b, h].reshape(NBLK, SBLK, D)
            qb = q[b, h].reshape(NBLK, SBLK, D)
            kb = k[b, h].reshape(NBLK, SBLK, D)
            S = np.zeros((2 * F, D), np.float32)
            Z = np.zeros((2 * F,), np.float32)
            for n in range(NBLK):
                den = np.maximum(pq[n] @ Z, EPS)
                lin = (pq[n] @ S) / den[:, None]
                S = S + pk[n].T @ vb[n]
                Z = Z + pk[n].sum(0)
                sc = qb[n] @ kb[n].T * SCALING
                p = _sm(sc)
                out[b, h, n * SBLK:(n + 1) * SBLK] = (
                    w * (p @ vb[n]) + (1 - w) * lin)
    return out


def _sm(x):
    e = np.exp(x - x.max(-1, keepdims=True))
    return e / e.sum(-1, keepdims=True)


# revision 19
# speedup vs baseline: 1.4079x; 1.4079x over previous
"""Trainium2 Bass kernel for BlockSoftmaxLinearHybrid.

The warm-call wall time is dominated by the axon tunnel (~78MB/s up,
~55MB/s down, full duplex), so the host<->device path is organized to
move the minimum number of bytes and overlap transfers:

  * inputs ship in natural (L, D) layout as bf16 -- no host transposes;
    the kernel transposes Q/K on device with the DMA XBAR and pads V
    with a ones column on device.
  * output ships as fp16 (error budget allows it).
  * the jitted shard_map dispatch and the zero output operands are
    cached across calls (zeros are not donated; the kernel writes every
    output element, so the result buffer needs no pre-zeroing).
  * work is split into NCALL chunks of (B*H)/NCALL pairs, pipelined so
    chunk j's download overlaps chunk j+1's upload.

Device kernel per (b,h) pair:
  phase A: u_q^T = W^T Q^T (f-major), EXPQ=[exp(u);exp(-u)] unnormalized
           (normalization recovered via ones-column in the state matmul);
           u_k in natural layout, exp'd and row-normalized -> phi_k.
  phase B: per 64-row block scan: block-local softmax attention
           (scores^T -> exp -> @[v|1]) + linear attention vs the running
           [S|Z] state accumulated in PSUM, blended with w=sigmoid(alpha).
"""

import sys

import numpy as np

if "/opt/trn_rl_repo" not in sys.path:
    sys.path.insert(0, "/opt/trn_rl_repo")

import ml_dtypes

import concourse.bass as bass
import concourse.bacc as bacc
import concourse.mybir as mybir
from concourse.tile import TileContext

B, H, L, D = 2, 16, 4096, 128
F = 64          # feature dim; phi dim is 2F = 128
SBLK = 64       # block size
NBLK = L // SBLK            # 64 blocks
NCH = L // 128              # 32 chunks (2 blocks each)
EPS = 1e-6
SCALING = D ** -0.5
NCORES = 8
PAIRS = B * H               # 32 (b,h) pairs
NCALL = 2                   # pipeline chunks per kernel() call
RPC = PAIRS // NCALL        # pairs per chunk (rows of the global array)
PPC = RPC // NCORES         # pairs per core per chunk
NGRP = NCH                  # phase-B group count

BF16 = mybir.dt.bfloat16
F16 = mybir.dt.float16
F32 = mybir.dt.float32
INT8 = mybir.dt.int8
AX = mybir.AxisListType
ALU = mybir.AluOpType
ACTF = mybir.ActivationFunctionType


def _bcast_last(ap, n):
    """Append a stride-0 dim of size n to an AP (free-dim broadcast)."""
    return bass.AP(tensor=ap.tensor, offset=ap.offset, ap=list(ap.ap) + [[0, n]])


def build_nc(w: float) -> bass.Bass:
    nc = bacc.Bacc()

    q_d = nc.dram_tensor("q", [PPC, L, D], BF16, kind="ExternalInput")
    # k/v rows quantized to int8 with a per-row bf16 scale packed in the
    # trailing 2 bytes (halves their upload bytes; v error is averaged
    # away by attention, k error only perturbs scores once -- q stays
    # bf16 because quantizing both sides of the score dot product would
    # double the error).
    kv_d = nc.dram_tensor("kv", [2, PPC, L, 130], INT8, kind="ExternalInput")
    wh_d = nc.dram_tensor("wh", [PPC, 128, F], BF16, kind="ExternalInput")
    out_d = nc.dram_tensor("out", [PPC, NCH, 128, D], F16, kind="ExternalOutput")

    with TileContext(nc) as tc:
        with (
            tc.tile_pool(name="sb", bufs=2) as sb,
            tc.tile_pool(name="small", bufs=2) as small,
            tc.tile_pool(name="const", bufs=1) as const,
            tc.tile_pool(name="grp", bufs=3) as grp,
            tc.tile_pool(name="pA", bufs=1, space="PSUM") as pA,
            tc.tile_pool(name="pSO", bufs=1, space="PSUM") as pSO,
            tc.tile_pool(name="pLQ", bufs=1, space="PSUM") as pLQ,
            tc.tile_pool(name="pST", bufs=2, space="PSUM") as pST,
        ):
            # identity matrix for PE-based 128x128 transposes of k
            ii = const.tile([128, 128], mybir.dt.int16, tag="ii")
            nc.gpsimd.iota(ii, pattern=[[1, 128]], channel_multiplier=-1)
            ident = const.tile([128, 128], BF16, tag="ident")
            nc.gpsimd.tensor_scalar(
                out=ident, in0=ii, scalar1=0, scalar2=None, op0=ALU.is_equal)

            for i in range(PPC):
                # ---- load pair inputs (Q transposed by the DMA XBAR) ----
                qt = sb.tile([128, L], BF16, tag="qt")
                nc.sync.dma_start_transpose(out=qt, in_=q_d[i])
                kai = sb.tile([128, NCH, 130], INT8, tag="kai")
                nc.sync.dma_start(
                    out=kai,
                    in_=kv_d[0][i].rearrange("(c p) k -> p c k", p=128))
                vai = sb.tile([128, NCH, 130], INT8, tag="vai")
                nc.sync.dma_start(
                    out=vai,
                    in_=kv_d[1][i].rearrange("(c p) k -> p c k", p=128))
                whs = small.tile([128, F], BF16, tag="wh")
                nc.sync.dma_start(out=whs, in_=wh_d[i])

                # dequantize v into [v|1] layout
                va = sb.tile([128, NCH, 130], BF16, tag="va")
                nc.vector.memset(va[:, :, 128:129], 1.0)
                vsc = vai[:, :, 128:130].bitcast(BF16)
                nc.vector.scalar_tensor_tensor(
                    va[:, :, 0:128], vai[:, :, 0:128], 1.0,
                    _bcast_last(vsc[:, :, 0], 128),
                    op0=ALU.mult, op1=ALU.mult)

                # dequantize k (natural layout), then transpose chunks on
                # the PE array to build kt (D-major)
                knb = sb.tile([128, NCH, 128], BF16, tag="knb")
                ksc = kai[:, :, 128:130].bitcast(BF16)
                nc.vector.scalar_tensor_tensor(
                    knb, kai[:, :, 0:128], 1.0,
                    _bcast_last(ksc[:, :, 0], 128),
                    op0=ALU.mult, op1=ALU.mult)
                kt = sb.tile([128, L], BF16, tag="kt")
                for c4 in range(8):
                    pT = pA.tile([128, 512], BF16, tag="mmT")
                    for cc in range(4):
                        c = c4 * 4 + cc
                        nc.tensor.transpose(
                            pT[:, cc * 128:(cc + 1) * 128], knb[:, c, :], ident)
                    nc.scalar.copy(kt[:, c4 * 512:(c4 + 1) * 512], pT)

                expq = sb.tile([128, L], BF16, tag="expq")
                expk = sb.tile([128, NCH, 128], BF16, tag="expk")
                phik = sb.tile([128, NCH, 128], BF16, tag="phik")
                outst = sb.tile([128, NCH, D], F16, tag="outst")

                # ---- phase A: q features (f-major, unnormalized) ----
                for j in range(8):
                    pu = pA.tile([128, 512], F32, tag="mm")
                    nc.tensor.matmul(
                        pu[0:64, :], lhsT=whs, rhs=qt[:, j * 512:(j + 1) * 512],
                        start=True, stop=True,
                    )
                    nc.scalar.activation(
                        expq[0:64, j * 512:(j + 1) * 512], pu[0:64, :], ACTF.Exp)
                    nc.scalar.activation(
                        expq[64:128, j * 512:(j + 1) * 512], pu[0:64, :], ACTF.Exp,
                        scale=-1.0)

                # ---- phase A: k features (natural layout) ----
                for jj in range(4):
                    pk = pA.tile([128, 512], F32, tag="mm")
                    for c8 in range(8):
                        c = jj * 8 + c8
                        nc.tensor.matmul(
                            pk[:, c8 * 64:(c8 + 1) * 64],
                            lhsT=kt[:, c * 128:(c + 1) * 128], rhs=whs,
                            start=True, stop=True,
                        )
                    pk3 = pk.rearrange("p (c f) -> p c f", f=64)
                    nc.scalar.activation(
                        expk[:, jj * 8:(jj + 1) * 8, 0:64], pk3, ACTF.Exp)
                    nc.scalar.activation(
                        expk[:, jj * 8:(jj + 1) * 8, 64:128], pk3, ACTF.Exp,
                        scale=-1.0)

                # normalize phi_k rows (per 64-feature half)
                sums = small.tile([128, NCH, 2], F32, tag="sums")
                nc.vector.tensor_reduce(
                    sums, expk.rearrange("p c (t f) -> p c t f", f=64),
                    axis=AX.X, op=ALU.add)
                recs = small.tile([128, NCH, 2], F32, tag="recs")
                nc.vector.reciprocal(recs, sums)
                for c in range(NCH):
                    for t in range(2):
                        nc.vector.tensor_scalar_mul(
                            phik[:, c, t * 64:(t + 1) * 64],
                            expk[:, c, t * 64:(t + 1) * 64],
                            recs[:, c, t:t + 1])

                # ---- phase B: block scan ----
                state = small.tile([128, 130], BF16, tag="state")
                nc.vector.memset(state[:, 0:129], 0.0)
                nc.vector.memset(state[:, 129:130], 1.0)
                sps_t = pST.tile([128, 512], F32, tag="st")
                sps = sps_t[:, 0:129]

                for g in range(NGRP):
                    c0, c1 = g * 128, (g + 1) * 128
                    # block-pair scores^T and exp
                    psc = pA.tile([128, 512], F32, tag="mm")
                    nc.tensor.matmul(
                        psc[:, 0:128], lhsT=kt[:, c0:c1], rhs=qt[:, c0:c1],
                        start=True, stop=True)
                    sst = grp.tile([128, 128], BF16, tag="sst")
                    nc.scalar.activation(sst, psc[:, 0:128], ACTF.Exp, scale=SCALING)

                    pso_t = pSO.tile([128, 512], F32, tag="so")
                    pso = pso_t[:, 0:129]
                    plq1_t = pLQ.tile([128, 512], F32, tag="lq1")
                    plq1 = plq1_t[:, 0:130]
                    plq2_t = pLQ.tile([128, 512], F32, tag="lq2")
                    plq2 = plq2_t[:, 0:130]

                    for h in range(2):  # even / odd block in the chunk
                        r0, r1 = h * 64, h * 64 + 64
                        # in-block softmax numerator @ [v|1]
                        nc.tensor.matmul(
                            pso[r0:r1, :], lhsT=sst[r0:r1, r0:r1],
                            rhs=va[r0:r1, g, 0:129],
                            start=True, stop=True, tile_position=(r0, r0))
                        # linear attention vs state (E and R halves)
                        nc.tensor.matmul(
                            plq1[r0:r1, 0:130],
                            lhsT=expq[0:64, c0 + h * 64: c0 + h * 64 + 64],
                            rhs=state[0:64, :],
                            start=True, stop=True, tile_position=(0, r0))
                        nc.tensor.matmul(
                            plq2[r0:r1, 0:130],
                            lhsT=expq[64:128, c0 + h * 64: c0 + h * 64 + 64],
                            rhs=state[64:128, :],
                            start=True, stop=True, tile_position=(64, r0))
                        # state update S += phi_k^T [v|1]
                        nc.tensor.matmul(
                            sps, lhsT=phik[r0:r1, g, :], rhs=va[r0:r1, g, 0:129],
                            start=(g == 0 and h == 0),
                            stop=(g == NGRP - 1 and h == 1),
                            skip_group_check=True,
                            tile_position=(r0, 0))
                        # refresh SBUF state copy for the next block
                        if not (g == NGRP - 1 and h == 1):
                            nc.scalar.copy(state[:, 0:129], sps)

                    # ---- assembly for the two blocks of this chunk ----
                    rs = grp.tile([128, 6], F32, tag="rs")
                    den = grp.tile([128, 2], F32, tag="den")
                    sc = grp.tile([128, 5], F32, tag="sc")
                    soev = grp.tile([128, 129], F32, tag="soev")
                    nc.scalar.copy(soev, pso)
                    lqev = grp.tile([128, 260], F32, tag="lqev")
                    nc.scalar.copy(lqev[:, 0:130], plq1)
                    nc.scalar.copy(lqev[:, 130:260], plq2)
                    nc.scalar.copy(sc[:, 0:1], soev[:, 128:129])
                    nc.scalar.copy(sc[:, 1:3], lqev[:, 128:130])
                    nc.scalar.copy(sc[:, 3:5], lqev[:, 258:260])
                    nc.vector.reciprocal(rs[:, 0:1], sc[:, 0:1])
                    nc.vector.reciprocal(rs[:, 1:2], sc[:, 2:3])
                    nc.vector.reciprocal(rs[:, 2:3], sc[:, 4:5])
                    nc.vector.tensor_scalar_mul(den[:, 0:1], sc[:, 1:2],
                                                rs[:, 1:2])
                    nc.vector.scalar_tensor_tensor(
                        den[:, 1:2], sc[:, 3:4], rs[:, 2:3], den[:, 0:1],
                        op0=ALU.mult, op1=ALU.add)
                    nc.vector.tensor_scalar_max(den[:, 0:1], den[:, 1:2], EPS)
                    nc.vector.reciprocal(rs[:, 3:4], den[:, 0:1])
                    nc.vector.tensor_scalar_mul(rs[:, 4:5], rs[:, 3:4], 1.0 - w)
                    nc.vector.tensor_scalar_mul(rs[:, 5:6], rs[:, 0:1], w)

                    t2 = grp.tile([128, 128], F32, tag="t2")
                    nc.vector.tensor_scalar_mul(t2, lqev[:, 0:128], rs[:, 1:2])
                    lin = grp.tile([128, 128], F32, tag="lin")
                    nc.vector.scalar_tensor_tensor(
                        lin, lqev[:, 130:258], rs[:, 2:3], t2,
                        op0=ALU.mult, op1=ALU.add)
                    sofl = grp.tile([128, 128], F32, tag="sofl")
                    nc.vector.tensor_scalar_mul(sofl, soev[:, 0:128], rs[:, 5:6])
                    nc.vector.scalar_tensor_tensor(
                        outst[:, g, :], lin, rs[:, 4:5], sofl,
                        op0=ALU.mult, op1=ALU.add)

                nc.sync.dma_start(out=out_d[i].rearrange("c p e -> p c e"),
                                  in_=outst)

    nc.compile()
    return nc


_STATE = {}


def _build_state(w: float):
    import jax
    from jax.sharding import Mesh, PartitionSpec, NamedSharding
    from jax.experimental.shard_map import shard_map
    from concourse.bass2jax import (
        _bass_exec_p, install_neuronx_cc_hook, partition_id_tensor)

    nc = build_nc(w)
    install_neuronx_cc_hook()

    partition_name = (
        nc.partition_id_tensor.name if nc.partition_id_tensor else None)
    in_names, out_names, out_avals = [], [], []
    for alloc in nc.m.functions[0].allocations:
        if not isinstance(alloc, mybir.MemoryLocationSet):
            continue
        name = alloc.memorylocations[0].name
        if alloc.kind == "ExternalInput":
            if name != partition_name:
                in_names.append(name)
        elif alloc.kind == "ExternalOutput":
            out_names.append(name)
            out_avals.append(jax.core.ShapedArray(
                tuple(alloc.tensor_shape), mybir.dt.np(alloc.dtype)))
    assert in_names == ["q", "kv", "wh"], in_names
    assert out_names == ["out"], out_names
    n_params = len(in_names)
    n_outs = len(out_names)
    all_in_names = list(in_names) + list(out_names)
    if partition_name is not None:
        all_in_names.append(partition_name)

    def _body(*args):
        operands = list(args)
        if partition_name is not None:
            operands.append(partition_id_tensor())
        outs = _bass_exec_p.bind(
            *operands,
            out_avals=tuple(out_avals),
            in_names=tuple(all_in_names),
            out_names=tuple(out_names),
            lowering_input_output_aliases=(),
            sim_require_finite=True,
            sim_require_nnan=True,
            nc=nc,
        )
        return tuple(outs)

    devices = jax.devices()[:NCORES]
    mesh = Mesh(np.asarray(devices), ("core",))
    spec = NamedSharding(mesh, PartitionSpec("core"))
    fn = jax.jit(
        shard_map(_body, mesh=mesh,
                  in_specs=(PartitionSpec("core"),) * (n_params + n_outs),
                  out_specs=(PartitionSpec("core"),) * n_outs,
                  check_rep=False),
        keep_unused=True,
    )
    # Output operands exist only to satisfy the HLO signature (the NEFF's
    # result buffers are bound separately and every element is written by
    # the kernel), so one cached device-resident zero array serves every
    # call with no per-call upload.
    zeros = [
        jax.device_put(
            np.zeros((NCORES * a.shape[0], *a.shape[1:]), a.dtype), spec)
        for a in out_avals
    ]
    return {"fn": fn, "spec": spec, "zeros": zeros}


def _get_state(w: float):
    key = round(w, 10)
    if key not in _STATE:
        _STATE[key] = _build_state(w)
    return _STATE[key]


def kernel(query_states, key_states, value_states, hedgehog_weights, alpha):
    q = np.asarray(query_states, dtype=np.float32)
    k = np.asarray(key_states, dtype=np.float32)
    v = np.asarray(value_states, dtype=np.float32)
    wts = np.asarray(hedgehog_weights, dtype=np.float32)
    a = float(np.asarray(alpha))
    w = float(1.0 / (1.0 + np.exp(-a)))

    try:
        return _run_device(q, k, v, wts, w)
    except Exception:
        import traceback
        traceback.print_exc(file=sys.stderr)
        return _host_reference(q, k, v, wts, w)


def _pack_q(q, j):
    bf = ml_dtypes.bfloat16
    sl = slice(j * RPC, (j + 1) * RPC)
    return np.asarray(q[sl], dtype=bf)


def _pack_kv(k, v, j):
    """Quantize k/v rows to int8 + per-row bf16 scale (trailing 2 bytes)."""
    bf = ml_dtypes.bfloat16
    sl = slice(j * RPC, (j + 1) * RPC)
    pack = np.empty((NCORES, 2, PPC, L, 130), dtype=np.int8)
    for t, x in enumerate((k, v)):
        xx = x[sl]                                 # fp32 [RPC, L, D]
        s = (np.abs(xx).max(-1) / 127.0 + 1e-30).astype(bf)
        inv = 1.0 / s.astype(np.float32)
        tq = xx * inv[..., None]
        np.rint(tq, out=tq)
        np.clip(tq, -127.0, 127.0, out=tq)
        dst = pack[:, t]                           # view [NCORES, PPC, L, 130]
        dst[:, :, :, 0:128] = tq.reshape(NCORES, PPC, L, D)
        dst[:, :, :, 128:130] = s.view(np.int8).reshape(NCORES, PPC, L, 2)
    return pack.reshape(NCORES * 2, PPC, L, 130)


def _run_device(q, k, v, wts, w):
    import jax
    from concurrent.futures import ThreadPoolExecutor

    st = _get_state(w)
    qf = q.reshape(PAIRS, L, D)
    kf = k.reshape(PAIRS, L, D)
    vf = v.reshape(PAIRS, L, D)
    # head index of global row r of chunk j is (j*RPC + r) % H: identical
    # for all chunks because RPC is a multiple of H (or H divides j*RPC).
    wh = np.asarray(
        wts[(np.arange(RPC)) % H], dtype=ml_dtypes.bfloat16)
    wh_dev = jax.device_put(wh.reshape(NCORES * PPC, 128, F), st["spec"])

    # pipeline: put_j / exec_j issued in order so chunk j's execution is
    # not queued behind chunk j+1's upload; fetches run on a worker thread
    # and overlap later uploads (the tunnel is full duplex).
    fetches = []
    with ThreadPoolExecutor(max_workers=2) as ex:
        for j in range(NCALL):
            q_dev = jax.device_put(_pack_q(qf, j), st["spec"])
            kv_dev = jax.device_put(_pack_kv(kf, vf, j), st["spec"])
            o = st["fn"](q_dev, kv_dev, wh_dev, *st["zeros"])[0]
            fetches.append(ex.submit(np.asarray, o))
        parts = [f.result() for f in fetches]

    res = np.empty((PAIRS, NCH, 128, D), dtype=np.float32)
    for j, p in enumerate(parts):
        res[j * RPC:(j + 1) * RPC] = p
    return res.reshape(B, H, L, D)


def _host_reference(q, k, v, wts, w):
    # Last-resort fallback so a transient device failure still returns
    # a correct result; mirrors the block-scan math in fp32 numpy.
    out = np.empty((B, H, L, D), dtype=np.float32)
    for b in range(B):
        for h in range(H):
            u = q[b, h].reshape(NBLK, SBLK, D) @ wts[h]
            pq = np.concatenate([_sm(u), _sm(-u)], -1)
            uk = k[b, h].reshape(NBLK, SBLK, D) @ wts[h]
            pk = np.concatenate([_sm(uk), _sm(-uk)], -1)
            vb = v[b, h].reshape(NBLK, SBLK, D)
            qb = q[b, h].reshape(NBLK, SBLK, D)
            kb = k[b, h].reshape(NBLK, SBLK, D)
            S = np.zeros((2 * F, D), np.float32)
            Z = np.zeros((2 * F,), np.float32)
            for n in range(NBLK):
                den = np.maximum(pq[n] @ Z, EPS)
                lin = (pq[n] @ S) / den[:, None]
                S = S + pk[n].T @ vb[n]
                Z = Z + pk[n].sum(0)
                sc = qb[n] @ kb[n].T * SCALING
                p = _sm(sc)
                out[b, h, n * SBLK:(n + 1) * SBLK] = (
                    w * (p @ vb[n]) + (1 - w) * lin)
    return out


def _sm(x):
    e = np.exp(x - x.max(-1, keepdims=True))
    return e / e.sum(-1, keepdims=True)


# revision 22
# speedup vs baseline: 1.6487x; 1.1710x over previous
"""Trainium2 Bass kernel for BlockSoftmaxLinearHybrid.

The warm-call wall time is dominated by the axon tunnel (~78MB/s up,
~55MB/s down, full duplex), so the host<->device path is organized to
move the minimum number of bytes and overlap transfers:

  * inputs ship in natural (L, D) layout as bf16 -- no host transposes;
    the kernel transposes Q/K on device with the DMA XBAR and pads V
    with a ones column on device.
  * output ships as fp16 (error budget allows it).
  * the jitted shard_map dispatch and the zero output operands are
    cached across calls (zeros are not donated; the kernel writes every
    output element, so the result buffer needs no pre-zeroing).
  * work is split into NCALL chunks of (B*H)/NCALL pairs, pipelined so
    chunk j's download overlaps chunk j+1's upload.

Device kernel per (b,h) pair:
  phase A: u_q^T = W^T Q^T (f-major), EXPQ=[exp(u);exp(-u)] unnormalized
           (normalization recovered via ones-column in the state matmul);
           u_k in natural layout, exp'd and row-normalized -> phi_k.
  phase B: per 64-row block scan: block-local softmax attention
           (scores^T -> exp -> @[v|1]) + linear attention vs the running
           [S|Z] state accumulated in PSUM, blended with w=sigmoid(alpha).
"""

import sys

import numpy as np

if "/opt/trn_rl_repo" not in sys.path:
    sys.path.insert(0, "/opt/trn_rl_repo")

import ml_dtypes

import concourse.bass as bass
import concourse.bacc as bacc
import concourse.mybir as mybir
from concourse.tile import TileContext

B, H, L, D = 2, 16, 4096, 128
F = 64          # feature dim; phi dim is 2F = 128
SBLK = 64       # block size
NBLK = L // SBLK            # 64 blocks
NCH = L // 128              # 32 chunks (2 blocks each)
EPS = 1e-6
SCALING = D ** -0.5
NCORES = 8
PAIRS = B * H               # 32 (b,h) pairs
NCALL = 4                   # pipeline chunks per kernel() call
RPC = PAIRS // NCALL        # pairs per chunk (rows of the global array)
PPC = RPC // NCORES         # pairs per core per chunk
NGRP = NCH                  # phase-B group count

BF16 = mybir.dt.bfloat16
F16 = mybir.dt.float16
F32 = mybir.dt.float32
INT8 = mybir.dt.int8
AX = mybir.AxisListType
ALU = mybir.AluOpType
ACTF = mybir.ActivationFunctionType


def _bcast_last(ap, n):
    """Append a stride-0 dim of size n to an AP (free-dim broadcast)."""
    return bass.AP(tensor=ap.tensor, offset=ap.offset, ap=list(ap.ap) + [[0, n]])


def build_nc(w: float) -> bass.Bass:
    nc = bacc.Bacc()

    q_d = nc.dram_tensor("q", [PPC, L, D], BF16, kind="ExternalInput")
    # k/v rows quantized to int8 with a per-row bf16 scale packed in the
    # trailing 2 bytes (halves their upload bytes; v error is averaged
    # away by attention, k error only perturbs scores once -- q stays
    # bf16 because quantizing both sides of the score dot product would
    # double the error).
    kv_d = nc.dram_tensor("kv", [2, PPC, L, 130], INT8, kind="ExternalInput")
    wh_d = nc.dram_tensor("wh", [PPC, 128, F], BF16, kind="ExternalInput")
    out_d = nc.dram_tensor("out", [PPC, NCH, 128, D], F16, kind="ExternalOutput")

    with TileContext(nc) as tc:
        with (
            tc.tile_pool(name="sb", bufs=2) as sb,
            tc.tile_pool(name="small", bufs=2) as small,
            tc.tile_pool(name="const", bufs=1) as const,
            tc.tile_pool(name="grp", bufs=3) as grp,
            tc.tile_pool(name="pA", bufs=1, space="PSUM") as pA,
            tc.tile_pool(name="pSO", bufs=1, space="PSUM") as pSO,
            tc.tile_pool(name="pLQ", bufs=1, space="PSUM") as pLQ,
            tc.tile_pool(name="pST", bufs=2, space="PSUM") as pST,
        ):
            # identity matrix for PE-based 128x128 transposes of k
            ii = const.tile([128, 128], mybir.dt.int16, tag="ii")
            nc.gpsimd.iota(ii, pattern=[[1, 128]], channel_multiplier=-1)
            ident = const.tile([128, 128], BF16, tag="ident")
            nc.gpsimd.tensor_scalar(
                out=ident, in0=ii, scalar1=0, scalar2=None, op0=ALU.is_equal)

            for i in range(PPC):
                # ---- load pair inputs (Q transposed by the DMA XBAR) ----
                qt = sb.tile([128, L], BF16, tag="qt")
                nc.sync.dma_start_transpose(out=qt, in_=q_d[i])
                kai = sb.tile([128, NCH, 130], INT8, tag="kai")
                nc.sync.dma_start(
                    out=kai,
                    in_=kv_d[0][i].rearrange("(c p) k -> p c k", p=128))
                vai = sb.tile([128, NCH, 130], INT8, tag="vai")
                nc.sync.dma_start(
                    out=vai,
                    in_=kv_d[1][i].rearrange("(c p) k -> p c k", p=128))
                whs = small.tile([128, F], BF16, tag="wh")
                nc.sync.dma_start(out=whs, in_=wh_d[i])

                # dequantize v into [v|1] layout
                va = sb.tile([128, NCH, 130], BF16, tag="va")
                nc.vector.memset(va[:, :, 128:129], 1.0)
                vsc = vai[:, :, 128:130].bitcast(BF16)
                nc.vector.scalar_tensor_tensor(
                    va[:, :, 0:128], vai[:, :, 0:128], 1.0,
                    _bcast_last(vsc[:, :, 0], 128),
                    op0=ALU.mult, op1=ALU.mult)

                # dequantize k (natural layout), then transpose chunks on
                # the PE array to build kt (D-major)
                knb = sb.tile([128, NCH, 128], BF16, tag="knb")
                ksc = kai[:, :, 128:130].bitcast(BF16)
                nc.vector.scalar_tensor_tensor(
                    knb, kai[:, :, 0:128], 1.0,
                    _bcast_last(ksc[:, :, 0], 128),
                    op0=ALU.mult, op1=ALU.mult)
                kt = sb.tile([128, L], BF16, tag="kt")
                for c4 in range(8):
                    pT = pA.tile([128, 512], BF16, tag="mmT")
                    for cc in range(4):
                        c = c4 * 4 + cc
                        nc.tensor.transpose(
                            pT[:, cc * 128:(cc + 1) * 128], knb[:, c, :], ident)
                    nc.scalar.copy(kt[:, c4 * 512:(c4 + 1) * 512], pT)

                expq = sb.tile([128, L], BF16, tag="expq")
                expk = sb.tile([128, NCH, 128], BF16, tag="expk")
                phik = sb.tile([128, NCH, 128], BF16, tag="phik")
                outst = sb.tile([128, NCH, D], F16, tag="outst")

                # ---- phase A: q features (f-major, unnormalized) ----
                for j in range(8):
                    pu = pA.tile([128, 512], F32, tag="mm")
                    nc.tensor.matmul(
                        pu[0:64, :], lhsT=whs, rhs=qt[:, j * 512:(j + 1) * 512],
                        start=True, stop=True,
                    )
                    nc.scalar.activation(
                        expq[0:64, j * 512:(j + 1) * 512], pu[0:64, :], ACTF.Exp)
                    nc.scalar.activation(
                        expq[64:128, j * 512:(j + 1) * 512], pu[0:64, :], ACTF.Exp,
                        scale=-1.0)

                # ---- phase A: k features (natural layout) ----
                for jj in range(4):
                    pk = pA.tile([128, 512], F32, tag="mm")
                    for c8 in range(8):
                        c = jj * 8 + c8
                        nc.tensor.matmul(
                            pk[:, c8 * 64:(c8 + 1) * 64],
                            lhsT=kt[:, c * 128:(c + 1) * 128], rhs=whs,
                            start=True, stop=True,
                        )
                    pk3 = pk.rearrange("p (c f) -> p c f", f=64)
                    nc.scalar.activation(
                        expk[:, jj * 8:(jj + 1) * 8, 0:64], pk3, ACTF.Exp)
                    nc.scalar.activation(
                        expk[:, jj * 8:(jj + 1) * 8, 64:128], pk3, ACTF.Exp,
                        scale=-1.0)

                # normalize phi_k rows (per 64-feature half)
                sums = small.tile([128, NCH, 2], F32, tag="sums")
                nc.vector.tensor_reduce(
                    sums, expk.rearrange("p c (t f) -> p c t f", f=64),
                    axis=AX.X, op=ALU.add)
                recs = small.tile([128, NCH, 2], F32, tag="recs")
                nc.vector.reciprocal(recs, sums)
                for c in range(NCH):
                    for t in range(2):
                        nc.vector.tensor_scalar_mul(
                            phik[:, c, t * 64:(t + 1) * 64],
                            expk[:, c, t * 64:(t + 1) * 64],
                            recs[:, c, t:t + 1])

                # ---- phase B: block scan ----
                state = small.tile([128, 130], BF16, tag="state")
                nc.vector.memset(state[:, 0:129], 0.0)
                nc.vector.memset(state[:, 129:130], 1.0)
                sps_t = pST.tile([128, 512], F32, tag="st")
                sps = sps_t[:, 0:129]

                for g in range(NGRP):
                    c0, c1 = g * 128, (g + 1) * 128
                    # block-pair scores^T and exp
                    psc = pA.tile([128, 512], F32, tag="mm")
                    nc.tensor.matmul(
                        psc[:, 0:128], lhsT=kt[:, c0:c1], rhs=qt[:, c0:c1],
                        start=True, stop=True)
                    sst = grp.tile([128, 128], BF16, tag="sst")
                    nc.scalar.activation(sst, psc[:, 0:128], ACTF.Exp, scale=SCALING)

                    pso_t = pSO.tile([128, 512], F32, tag="so")
                    pso = pso_t[:, 0:129]
                    plq1_t = pLQ.tile([128, 512], F32, tag="lq1")
                    plq1 = plq1_t[:, 0:130]
                    plq2_t = pLQ.tile([128, 512], F32, tag="lq2")
                    plq2 = plq2_t[:, 0:130]

                    for h in range(2):  # even / odd block in the chunk
                        r0, r1 = h * 64, h * 64 + 64
                        # in-block softmax numerator @ [v|1]
                        nc.tensor.matmul(
                            pso[r0:r1, :], lhsT=sst[r0:r1, r0:r1],
                            rhs=va[r0:r1, g, 0:129],
                            start=True, stop=True, tile_position=(r0, r0))
                        # linear attention vs state (E and R halves)
                        nc.tensor.matmul(
                            plq1[r0:r1, 0:130],
                            lhsT=expq[0:64, c0 + h * 64: c0 + h * 64 + 64],
                            rhs=state[0:64, :],
                            start=True, stop=True, tile_position=(0, r0))
                        nc.tensor.matmul(
                            plq2[r0:r1, 0:130],
                            lhsT=expq[64:128, c0 + h * 64: c0 + h * 64 + 64],
                            rhs=state[64:128, :],
                            start=True, stop=True, tile_position=(64, r0))
                        # state update S += phi_k^T [v|1]
                        nc.tensor.matmul(
                            sps, lhsT=phik[r0:r1, g, :], rhs=va[r0:r1, g, 0:129],
                            start=(g == 0 and h == 0),
                            stop=(g == NGRP - 1 and h == 1),
                            skip_group_check=True,
                            tile_position=(r0, 0))
                        # refresh SBUF state copy for the next block
                        if not (g == NGRP - 1 and h == 1):
                            nc.scalar.copy(state[:, 0:129], sps)

                    # ---- assembly for the two blocks of this chunk ----
                    rs = grp.tile([128, 6], F32, tag="rs")
                    den = grp.tile([128, 2], F32, tag="den")
                    sc = grp.tile([128, 5], F32, tag="sc")
                    soev = grp.tile([128, 129], F32, tag="soev")
                    nc.scalar.copy(soev, pso)
                    lqev = grp.tile([128, 260], F32, tag="lqev")
                    nc.scalar.copy(lqev[:, 0:130], plq1)
                    nc.scalar.copy(lqev[:, 130:260], plq2)
                    nc.scalar.copy(sc[:, 0:1], soev[:, 128:129])
                    nc.scalar.copy(sc[:, 1:3], lqev[:, 128:130])
                    nc.scalar.copy(sc[:, 3:5], lqev[:, 258:260])
                    nc.vector.reciprocal(rs[:, 0:1], sc[:, 0:1])
                    nc.vector.reciprocal(rs[:, 1:2], sc[:, 2:3])
                    nc.vector.reciprocal(rs[:, 2:3], sc[:, 4:5])
                    nc.vector.tensor_scalar_mul(den[:, 0:1], sc[:, 1:2],
                                                rs[:, 1:2])
                    nc.vector.scalar_tensor_tensor(
                        den[:, 1:2], sc[:, 3:4], rs[:, 2:3], den[:, 0:1],
                        op0=ALU.mult, op1=ALU.add)
                    nc.vector.tensor_scalar_max(den[:, 0:1], den[:, 1:2], EPS)
                    nc.vector.reciprocal(rs[:, 3:4], den[:, 0:1])
                    nc.vector.tensor_scalar_mul(rs[:, 4:5], rs[:, 3:4], 1.0 - w)
                    nc.vector.tensor_scalar_mul(rs[:, 5:6], rs[:, 0:1], w)

                    t2 = grp.tile([128, 128], F32, tag="t2")
                    nc.vector.tensor_scalar_mul(t2, lqev[:, 0:128], rs[:, 1:2])
                    lin = grp.tile([128, 128], F32, tag="lin")
                    nc.vector.scalar_tensor_tensor(
                        lin, lqev[:, 130:258], rs[:, 2:3], t2,
                        op0=ALU.mult, op1=ALU.add)
                    sofl = grp.tile([128, 128], F32, tag="sofl")
                    nc.vector.tensor_scalar_mul(sofl, soev[:, 0:128], rs[:, 5:6])
                    nc.vector.scalar_tensor_tensor(
                        outst[:, g, :], lin, rs[:, 4:5], sofl,
                        op0=ALU.mult, op1=ALU.add)

                nc.sync.dma_start(out=out_d[i].rearrange("c p e -> p c e"),
                                  in_=outst)

    nc.compile()
    return nc


_STATE = {}


def _build_state(w: float):
    import jax
    from jax.sharding import Mesh, PartitionSpec, NamedSharding
    from jax.experimental.shard_map import shard_map
    from concourse.bass2jax import (
        _bass_exec_p, install_neuronx_cc_hook, partition_id_tensor)

    nc = build_nc(w)
    install_neuronx_cc_hook()

    partition_name = (
        nc.partition_id_tensor.name if nc.partition_id_tensor else None)
    in_names, out_names, out_avals = [], [], []
    for alloc in nc.m.functions[0].allocations:
        if not isinstance(alloc, mybir.MemoryLocationSet):
            continue
        name = alloc.memorylocations[0].name
        if alloc.kind == "ExternalInput":
            if name != partition_name:
                in_names.append(name)
        elif alloc.kind == "ExternalOutput":
            out_names.append(name)
            out_avals.append(jax.core.ShapedArray(
                tuple(alloc.tensor_shape), mybir.dt.np(alloc.dtype)))
    assert in_names == ["q", "kv", "wh"], in_names
    assert out_names == ["out"], out_names
    n_params = len(in_names)
    n_outs = len(out_names)
    all_in_names = list(in_names) + list(out_names)
    if partition_name is not None:
        all_in_names.append(partition_name)

    def _body(*args):
        operands = list(args)
        if partition_name is not None:
            operands.append(partition_id_tensor())
        outs = _bass_exec_p.bind(
            *operands,
            out_avals=tuple(out_avals),
            in_names=tuple(all_in_names),
            out_names=tuple(out_names),
            lowering_input_output_aliases=(),
            sim_require_finite=True,
            sim_require_nnan=True,
            nc=nc,
        )
        return tuple(outs)

    devices = jax.devices()[:NCORES]
    mesh = Mesh(np.asarray(devices), ("core",))
    spec = NamedSharding(mesh, PartitionSpec("core"))
    fn = jax.jit(
        shard_map(_body, mesh=mesh,
                  in_specs=(PartitionSpec("core"),) * (n_params + n_outs),
                  out_specs=(PartitionSpec("core"),) * n_outs,
                  check_rep=False),
        keep_unused=True,
    )
    # Output operands exist only to satisfy the HLO signature (the NEFF's
    # result buffers are bound separately and every element is written by
    # the kernel), so one cached device-resident zero array serves every
    # call with no per-call upload.
    zeros = [
        jax.device_put(
            np.zeros((NCORES * a.shape[0], *a.shape[1:]), a.dtype), spec)
        for a in out_avals
    ]
    return {"fn": fn, "spec": spec, "zeros": zeros}


def _get_state(w: float):
    key = round(w, 10)
    if key not in _STATE:
        _STATE[key] = _build_state(w)
    return _STATE[key]


def kernel(query_states, key_states, value_states, hedgehog_weights, alpha):
    q = np.asarray(query_states, dtype=np.float32)
    k = np.asarray(key_states, dtype=np.float32)
    v = np.asarray(value_states, dtype=np.float32)
    wts = np.asarray(hedgehog_weights, dtype=np.float32)
    a = float(np.asarray(alpha))
    w = float(1.0 / (1.0 + np.exp(-a)))

    try:
        return _run_device(q, k, v, wts, w)
    except Exception:
        import traceback
        traceback.print_exc(file=sys.stderr)
        return _host_reference(q, k, v, wts, w)


def _pack_q(q, j):
    bf = ml_dtypes.bfloat16
    sl = slice(j * RPC, (j + 1) * RPC)
    return np.asarray(q[sl], dtype=bf)


def _pack_kv(k, v, j):
    """Quantize k/v rows to int8 + per-row bf16 scale (trailing 2 bytes)."""
    bf = ml_dtypes.bfloat16
    sl = slice(j * RPC, (j + 1) * RPC)
    pack = np.empty((NCORES, 2, PPC, L, 130), dtype=np.int8)
    for t, x in enumerate((k, v)):
        xx = x[sl]                                 # fp32 [RPC, L, D]
        s = (np.abs(xx).max(-1) / 127.0 + 1e-30).astype(bf)
        inv = 1.0 / s.astype(np.float32)
        tq = xx * inv[..., None]
        np.rint(tq, out=tq)
        np.clip(tq, -127.0, 127.0, out=tq)
        dst = pack[:, t]                           # view [NCORES, PPC, L, 130]
        dst[:, :, :, 0:128] = tq.reshape(NCORES, PPC, L, D)
        dst[:, :, :, 128:130] = s.view(np.int8).reshape(NCORES, PPC, L, 2)
    return pack.reshape(NCORES * 2, PPC, L, 130)


def _run_device(q, k, v, wts, w):
    import math

    import jax
    from concurrent.futures import ThreadPoolExecutor

    st = _get_state(w)
    qf = q.reshape(PAIRS, L, D)
    kf = k.reshape(PAIRS, L, D)
    vf = v.reshape(PAIRS, L, D)
    # head index of global row r of chunk j is (j*RPC + r) % H, which
    # repeats with period H//gcd(H, RPC) in j; upload one small wh array
    # per distinct pattern and reuse across chunks.
    period = H // math.gcd(H, RPC)
    wh_devs = [
        jax.device_put(
            np.asarray(wts[(j * RPC + np.arange(RPC)) % H],
                       dtype=ml_dtypes.bfloat16),
            st["spec"])
        for j in range(period)
    ]

    # pipeline: put_j / exec_j issued in order so chunk j's execution is
    # not queued behind chunk j+1's upload; fetches run on a worker thread
    # and overlap later uploads (the tunnel is full duplex).
    fetches = []
    with ThreadPoolExecutor(max_workers=2) as ex:
        for j in range(NCALL):
            q_dev = jax.device_put(_pack_q(qf, j), st["spec"])
            kv_dev = jax.device_put(_pack_kv(kf, vf, j), st["spec"])
            o = st["fn"](q_dev, kv_dev, wh_devs[j % period], *st["zeros"])[0]
            fetches.append(ex.submit(np.asarray, o))
        parts = [f.result() for f in fetches]

    res = np.empty((PAIRS, NCH, 128, D), dtype=np.float32)
    for j, p in enumerate(parts):
        res[j * RPC:(j + 1) * RPC] = p
    return res.reshape(B, H, L, D)


def _host_reference(q, k, v, wts, w):
    # Last-resort fallback so a transient device failure still returns
    # a correct result; mirrors the block-scan math in fp32 numpy.
    out = np.empty((B, H, L, D), dtype=np.float32)
    for b in range(B):
        for h in range(H):
            u = q[b, h].reshape(NBLK, SBLK, D) @ wts[h]
            pq = np.concatenate([_sm(u), _sm(-u)], -1)
            uk = k[b, h].reshape(NBLK, SBLK, D) @ wts[h]
            pk = np.concatenate([_sm(uk), _sm(-uk)], -1)
            vb = v[b, h].reshape(NBLK, SBLK, D)
            qb = q[b, h].reshape(NBLK, SBLK, D)
            kb = k[b, h].reshape(NBLK, SBLK, D)
            S = np.zeros((2 * F, D), np.float32)
            Z = np.zeros((2 * F,), np.float32)
            for n in range(NBLK):
                den = np.maximum(pq[n] @ Z, EPS)
                lin = (pq[n] @ S) / den[:, None]
                S = S + pk[n].T @ vb[n]
                Z = Z + pk[n].sum(0)
                sc = qb[n] @ kb[n].T * SCALING
                p = _sm(sc)
                out[b, h, n * SBLK:(n + 1) * SBLK] = (
                    w * (p @ vb[n]) + (1 - w) * lin)
    return out


def _sm(x):
    e = np.exp(x - x.max(-1, keepdims=True))
    return e / e.sum(-1, keepdims=True)
